# revision 19
# baseline (speedup 1.0000x reference)
"""Trainium2 Bass kernel for nn_AdaptiveAlphaQuantizedLinear.

out[b,t,k] = sum_n x[b,t,n]*mu1[n] * ((W_q[k,n]-zeros[k,g(n)])*scales[k,g(n)])*mu2[k]
             + bias[k]

Strategy (8 NeuronCores, tensor-parallel along K), v5:
  Host prep:
    - a[k,g] = scales*mu2 and c[k,g] = -zeros*scales*mu2 folded host-side.
    - x' = x*mu1; group sums Xg and a ones row appended as 65 extra
      contraction rows so the zeros+bias term rides one small matmul.
    - Contraction order INTERLEAVED: PE tile t, partition p holds original
      n = (p//2)*128 + 2t + (p%2).  Every 128-row tile then contains 2 rows
      of each quant group, so the dequant scale tile srep[p,k] = a[k, p//2]
      is IDENTICAL for all 64 tiles -> loaded once, no on-device scale
      replication.
    - Per 8-tile block, 3 tiles (t%8 in {0,3,6}) ship pre-dequantized bf16
      (no device work) and 5 ship as int8 codes (4x less DMA); the 3:5 mix
      keeps the DVE dequant rate ahead of PE consumption.
  Device per core:
    - W tiles stream per-tile on the SP sequencer's HWDGE queue; xt/srep/
      ct/xgt go on the ACT sequencer's queue so the SP issue serialization
      (~0.6us per dma_start) doesn't starve the PE at the head.
    - DVE dequants int8 tiles (mixed int8 x bf16 tensor_mul vs srep); PE
      runs 4 accumulating matmuls per tile back-to-back (p-state stays at
      max clock).
    - Xg/ones extra rows close the accumulation with the zeros/bias term.
    - ACT copies PSUM -> SBUF as bf16 (rel-err budget is 2e-2; bf16 round
      adds ~2e-3), DMA out [256, 1024] bf16, host upcasts to f32.
  host: concat k-shards, reshape to [8, 32, 8192].
"""
import sys
sys.path.insert(0, "/opt/trn_rl_repo")
import numpy as np

K = 8192
N = 8192
GROUP_SIZE = 128
NG = N // GROUP_SIZE          # 64 groups
B, T = 8, 32
BT = B * T                    # 256
NCORES = 8
KSH = K // NCORES             # 1024 out-features per core
NT = N // 128                 # 64 n-tiles
# 3:5 direct:int8 interleave per 8-tile block
IS_DIR = [t % 8 in (0, 3, 6) for t in range(NT)]
DIR_IDX = np.cumsum([0] + IS_DIR[:-1]).tolist()
I8_IDX = np.cumsum([0] + [not d for d in IS_DIR[:-1]]).tolist()
HT = sum(IS_DIR)              # 24 direct tiles, 40 int8 tiles

_NC_CACHE = None


def _build():
    from concourse import bacc, tile, mybir

    bf16 = mybir.dt.bfloat16
    nc = bacc.Bacc("TRN2", target_bir_lowering=False, debug=False,
                   num_devices=NCORES)
    wdir = nc.dram_tensor("wdir", [HT, 128, KSH], bf16, kind="ExternalInput")
    wq8 = nc.dram_tensor("wq8", [NT - HT, 128, KSH], mybir.dt.int8,
                         kind="ExternalInput")
    # xt pre-transposed host-side to partition-major chunks: each chunk DMA
    # is a contiguous [128, XCH*BT*2B] copy (big packets; the on-the-fly
    # "t p d -> p t d" rearrange shredded it into 512B packets)
    xt = nc.dram_tensor("xt", [NT // 8, 128, 8, BT], bf16,
                        kind="ExternalInput")
    srep = nc.dram_tensor("srep", [128, KSH], bf16, kind="ExternalInput")
    xgt = nc.dram_tensor("xgt", [NG + 1, BT], bf16, kind="ExternalInput")
    ct = nc.dram_tensor("ct", [NG + 1, KSH], bf16, kind="ExternalInput")
    out = nc.dram_tensor("out", [BT, KSH], bf16, kind="ExternalOutput")

    XCH = 8                   # xt tiles per DMA chunk
    NXC = NT // XCH           # 8 chunks
    LOOK = 12                 # W-tile DMA lookahead
    DQ = 7                    # dequant (DVE) lookahead over PE

    with tile.TileContext(nc) as tc:
        with (
            tc.tile_pool(name="const", bufs=1) as cpool,
            tc.tile_pool(name="wd", bufs=8) as wdpool,
            tc.tile_pool(name="wq", bufs=12) as wqpool,
            tc.tile_pool(name="ws", bufs=10) as wspool,
            tc.tile_pool(name="psum", bufs=1, space="PSUM") as psum,
            tc.tile_pool(name="outp", bufs=1) as opool,
        ):
            xt_sb = cpool.tile([128, NT, BT], bf16, tag="xt")
            srep_sb = cpool.tile([128, KSH], bf16, tag="srep")
            xg_sb = cpool.tile([NG + 1, BT], bf16, tag="xg")
            ct_sb = cpool.tile([NG + 1, KSH], bf16, tag="ct")

            def load_xt_chunk(c, lo=0):
                nc.scalar.dma_start(
                    xt_sb[:, c * XCH + lo:(c + 1) * XCH, :],
                    xt[c, :, lo:XCH, :])

            wtiles = {}

            def fetch_w(t):
                if IS_DIR[t]:
                    wd = wdpool.tile([128, KSH], bf16, tag="wd", name="wd")
                    nc.sync.dma_start(wd[:], wdir[DIR_IDX[t]])
                    wtiles[t] = wd
                else:
                    wq = wqpool.tile([128, KSH], mybir.dt.int8, tag="wq",
                                     name="wq")
                    nc.sync.dma_start(wq[:], wq8[I8_IDX[t]])
                    wtiles[t] = wq

            ws_ready = {}

            def dequant(t):
                # direct tiles pass through; int8 tiles get one DVE mul
                if IS_DIR[t]:
                    ws_ready[t] = wtiles.pop(t)
                else:
                    wq = wtiles.pop(t)
                    ws = wspool.tile([128, KSH], bf16, tag="ws", name="ws")
                    nc.vector.tensor_mul(ws[:], wq[:], srep_sb[:])
                    ws_ready[t] = ws

            # head: W stream starts immediately on SP; x/scales on ACT queue.
            # First x minichunk (2 tiles) + first W tile land in <1us so the
            # PE starts as early as possible.
            nc.scalar.dma_start(xt_sb[:, 0:2, :], xt[0, :, 0:2, :])
            fetch_w(0)
            nc.scalar.dma_start(srep_sb[:], srep[:])
            load_xt_chunk(0, lo=2)
            load_xt_chunk(1)
            for t in range(1, LOOK):
                fetch_w(t)
            for t in range(DQ):
                dequant(t)

            accs = [psum.tile([128, 512], mybir.dt.float32, tag=f"acc{b}{c}",
                              name=f"acc{b}{c}")
                    for b in range(2) for c in range(2)]

            # c-term + bias FIRST (start=True): out += Xg2[bt,g] @ cT[g,k].
            # Putting it at the head (it only needs the tiny xgt/ct loads)
            # removes it from the output critical path at the tail.
            nc.scalar.dma_start(xg_sb[:], xgt[:])
            nc.scalar.dma_start(ct_sb[:], ct[:])
            for b in range(2):
                for c in range(2):
                    nc.tensor.matmul(
                        accs[b * 2 + c][:],
                        xg_sb[:, b * 128:(b + 1) * 128],
                        ct_sb[:, c * 512:(c + 1) * 512],
                        start=True, stop=False,
                    )

            for t in range(NT):
                if t + LOOK < NT:
                    fetch_w(t + LOOK)
                if t + DQ < NT:
                    dequant(t + DQ)
                if t % XCH == 0 and t // XCH + 2 < NXC:
                    load_xt_chunk(t // XCH + 2)
                ws = ws_ready.pop(t)
                for b in range(2):
                    for c in range(2):
                        nc.tensor.matmul(
                            accs[b * 2 + c][:],
                            xt_sb[:, t, b * 128:(b + 1) * 128],
                            ws[:, c * 512:(c + 1) * 512],
                            start=False, stop=(t == NT - 1),
                        )

            out_sb = opool.tile([128, 2, KSH], bf16, tag="o")
            out_v = out.ap().rearrange("(b p) k -> p b k", p=128)
            for b in range(2):
                for c in range(2):
                    nc.scalar.copy(out_sb[:, b, c * 512:(c + 1) * 512],
                                   accs[b * 2 + c][:])
                    nc.sync.dma_start(
                        out_v[:, b, c * 512:(c + 1) * 512],
                        out_sb[:, b, c * 512:(c + 1) * 512])

    nc.compile()
    return nc


def _get_nc():
    global _NC_CACHE
    if _NC_CACHE is None:
        _NC_CACHE = _build()
    return _NC_CACHE


def _perm_index():
    # n_of[t, p] = original contraction index held by tile t, partition p
    t = np.arange(NT)[:, None]
    p = np.arange(128)[None, :]
    return (p // 2) * GROUP_SIZE + 2 * t + (p % 2)      # [NT, 128]


def _prep_in_maps(x, W_q, scales, zeros, mu1, mu2, bias):
    import ml_dtypes
    bf16 = ml_dtypes.bfloat16
    x2 = np.asarray(x, dtype=np.float32).reshape(BT, N)
    mu1 = np.asarray(mu1, dtype=np.float32)
    mu2 = np.asarray(mu2, dtype=np.float32)
    bias = np.asarray(bias, dtype=np.float32)
    sc = np.asarray(scales, dtype=np.float32)[:, :, 0]   # [K, NG]
    zr = np.asarray(zeros, dtype=np.float32)[:, :, 0]    # [K, NG]
    W_q = np.asarray(W_q)

    n_of = _perm_index()                                  # [NT, 128]

    xp = x2 * mu1[None, :]                                # x' [BT, N]
    # [NXC=8, 128, XCH=8, BT]: partition-major per chunk
    xt_h = np.ascontiguousarray(
        xp.T[n_of.reshape(-1)].reshape(NT // 8, 8, 128, BT)
        .transpose(0, 2, 1, 3)).astype(bf16)
    Xg = xp.reshape(BT, NG, GROUP_SIZE).sum(axis=2)       # [BT, NG]
    xgt_h = np.concatenate(
        [np.ascontiguousarray(Xg.T), np.ones((1, BT), np.float32)],
        axis=0).astype(bf16)                              # [NG+1, BT]

    a = sc * mu2[:, None]                                 # [K, NG]
    cmat = -zr * a                                        # [K, NG]
    g_of_p = np.arange(128) // 2                          # [128]
    dmask = np.asarray(IS_DIR)

    in_maps = []
    for i in range(NCORES):
        ksl = slice(i * KSH, (i + 1) * KSH)
        wq_core = W_q[ksl, :]                             # [KSH, N] int32
        # [NT, 128, KSH]: tile-major, interleaved rows
        wq_perm = wq_core.T[n_of.reshape(-1)].reshape(NT, 128, KSH)
        srep_h = np.ascontiguousarray(a[ksl, :].T[g_of_p, :]).astype(bf16)
        srep_f = srep_h.astype(np.float32)                # bf16-rounded scales
        wdir_h = np.ascontiguousarray(
            wq_perm[dmask].astype(np.float32) * srep_f[None, :, :]).astype(bf16)
        wq8_h = np.ascontiguousarray(wq_perm[~dmask].astype(np.int8))
        ct_h = np.concatenate(
            [np.ascontiguousarray(cmat[ksl, :].T),
             bias[None, ksl]], axis=0).astype(bf16)       # [NG+1, KSH]
        in_maps.append({"wdir": wdir_h, "wq8": wq8_h, "xt": xt_h,
                        "srep": srep_h, "xgt": xgt_h, "ct": ct_h})
    return in_maps


def _run(inputs, trace=False):
    from concourse import bass_utils
    nc = _get_nc()
    in_maps = _prep_in_maps(**inputs)
    res = bass_utils.run_bass_kernel_spmd(
        nc, in_maps, core_ids=list(range(NCORES)), trace=trace)
    out = np.concatenate([res.results[i]["out"] for i in range(NCORES)],
                         axis=1)                          # [BT, K]
    return out.reshape(B, T, K).astype(np.float32), res


def kernel(**inputs) -> np.ndarray:
    out, _ = _run(inputs, trace=False)
    return out


def kernel_traced(**inputs):
    out, res = _run(inputs, trace=True)
    return out, res


# revision 21
# speedup vs baseline: 1.0383x; 1.0383x over previous
"""Trainium2 Bass kernel for nn_AdaptiveAlphaQuantizedLinear.

out[b,t,k] = sum_n x[b,t,n]*mu1[n] * ((W_q[k,n]-zeros[k,g(n)])*scales[k,g(n)])*mu2[k]
             + bias[k]

Strategy (8 NeuronCores, tensor-parallel along K), v5:
  Host prep:
    - a[k,g] = scales*mu2 and c[k,g] = -zeros*scales*mu2 folded host-side.
    - x' = x*mu1; group sums Xg and a ones row appended as 65 extra
      contraction rows so the zeros+bias term rides one small matmul.
    - Contraction order INTERLEAVED: PE tile t, partition p holds original
      n = (p//2)*128 + 2t + (p%2).  Every 128-row tile then contains 2 rows
      of each quant group, so the dequant scale tile srep[p,k] = a[k, p//2]
      is IDENTICAL for all 64 tiles -> loaded once, no on-device scale
      replication.
    - Per 8-tile block, 3 tiles (t%8 in {0,3,6}) ship pre-dequantized bf16
      (no device work) and 5 ship as int8 codes (4x less DMA); the 3:5 mix
      keeps the DVE dequant rate ahead of PE consumption.
  Device per core:
    - W tiles stream per-tile on the SP sequencer's HWDGE queue; xt/srep/
      ct/xgt go on the ACT sequencer's queue so the SP issue serialization
      (~0.6us per dma_start) doesn't starve the PE at the head.
    - DVE dequants int8 tiles (mixed int8 x bf16 tensor_mul vs srep); PE
      runs 4 accumulating matmuls per tile back-to-back (p-state stays at
      max clock).
    - Xg/ones extra rows close the accumulation with the zeros/bias term.
    - ACT copies PSUM -> SBUF as bf16 (rel-err budget is 2e-2; bf16 round
      adds ~2e-3), DMA out [256, 1024] bf16, host upcasts to f32.
  host: concat k-shards, reshape to [8, 32, 8192].
"""
import sys
sys.path.insert(0, "/opt/trn_rl_repo")
import numpy as np

K = 8192
N = 8192
GROUP_SIZE = 128
NG = N // GROUP_SIZE          # 64 groups
B, T = 8, 32
BT = B * T                    # 256
NCORES = 8
KSH = K // NCORES             # 1024 out-features per core
NT = N // 128                 # 64 n-tiles
# 3:5 direct:int8 interleave per 8-tile block
IS_DIR = [t % 8 in (0, 3, 6) for t in range(NT)]
DIR_IDX = np.cumsum([0] + IS_DIR[:-1]).tolist()
I8_IDX = np.cumsum([0] + [not d for d in IS_DIR[:-1]]).tolist()
HT = sum(IS_DIR)              # 24 direct tiles, 40 int8 tiles

_NC_CACHE = None


def _build():
    from concourse import bacc, tile, mybir

    bf16 = mybir.dt.bfloat16
    nc = bacc.Bacc("TRN2", target_bir_lowering=False, debug=False,
                   num_devices=NCORES)
    wdir = nc.dram_tensor("wdir", [HT, 128, KSH], bf16, kind="ExternalInput")
    wq8 = nc.dram_tensor("wq8", [NT - HT, 128, KSH], mybir.dt.int8,
                         kind="ExternalInput")
    # xt pre-transposed host-side to partition-major chunks: each chunk DMA
    # is a contiguous [128, XCH*BT*2B] copy (big packets; the on-the-fly
    # "t p d -> p t d" rearrange shredded it into 512B packets)
    xt = nc.dram_tensor("xt", [NT // 8, 128, 8, BT], bf16,
                        kind="ExternalInput")
    srep = nc.dram_tensor("srep", [128, KSH], bf16, kind="ExternalInput")
    xgt = nc.dram_tensor("xgt", [NG + 1, BT], bf16, kind="ExternalInput")
    ct = nc.dram_tensor("ct", [NG + 1, KSH], bf16, kind="ExternalInput")
    out = nc.dram_tensor("out", [BT, KSH], bf16, kind="ExternalOutput")

    XCH = 8                   # xt tiles per DMA chunk
    NXC = NT // XCH           # 8 chunks
    LOOK = 12                 # W-tile DMA lookahead
    DQ = 7                    # dequant (DVE) lookahead over PE

    with tile.TileContext(nc) as tc:
        with (
            tc.tile_pool(name="const", bufs=1) as cpool,
            tc.tile_pool(name="wd", bufs=8) as wdpool,
            tc.tile_pool(name="wq", bufs=12) as wqpool,
            tc.tile_pool(name="ws", bufs=10) as wspool,
            tc.tile_pool(name="psum", bufs=1, space="PSUM") as psum,
            tc.tile_pool(name="outp", bufs=1) as opool,
        ):
            xt_sb = cpool.tile([128, NT, BT], bf16, tag="xt")
            srep_sb = cpool.tile([128, KSH], bf16, tag="srep")
            xg_sb = cpool.tile([NG + 1, BT], bf16, tag="xg")
            ct_sb = cpool.tile([NG + 1, KSH], bf16, tag="ct")

            def load_xt_chunk(c, lo=0):
                nc.scalar.dma_start(
                    xt_sb[:, c * XCH + lo:(c + 1) * XCH, :],
                    xt[c, :, lo:XCH, :])

            wtiles = {}

            def fetch_w(t):
                if IS_DIR[t]:
                    wd = wdpool.tile([128, KSH], bf16, tag="wd", name="wd")
                    nc.sync.dma_start(wd[:], wdir[DIR_IDX[t]])
                    wtiles[t] = wd
                else:
                    wq = wqpool.tile([128, KSH], mybir.dt.int8, tag="wq",
                                     name="wq")
                    nc.sync.dma_start(wq[:], wq8[I8_IDX[t]])
                    wtiles[t] = wq

            ws_ready = {}

            def dequant(t):
                # direct tiles pass through; int8 tiles get one DVE mul
                if IS_DIR[t]:
                    ws_ready[t] = wtiles.pop(t)
                else:
                    wq = wtiles.pop(t)
                    ws = wspool.tile([128, KSH], bf16, tag="ws", name="ws")
                    nc.vector.tensor_mul(ws[:], wq[:], srep_sb[:])
                    ws_ready[t] = ws

            # head: W stream starts immediately on SP; x/scales on ACT queue.
            # First x minichunk (2 tiles) + first W tile land in <1us so the
            # PE starts as early as possible.
            nc.scalar.dma_start(xt_sb[:, 0:2, :], xt[0, :, 0:2, :])
            fetch_w(0)
            nc.scalar.dma_start(srep_sb[:], srep[:])
            load_xt_chunk(0, lo=2)
            load_xt_chunk(1)
            for t in range(1, LOOK):
                fetch_w(t)
            for t in range(DQ):
                dequant(t)

            accs = [psum.tile([128, 512], mybir.dt.float32, tag=f"acc{b}{c}",
                              name=f"acc{b}{c}")
                    for b in range(2) for c in range(2)]

            nc.scalar.dma_start(xg_sb[:], xgt[:])
            nc.scalar.dma_start(ct_sb[:], ct[:])

            for t in range(NT):
                if t + LOOK < NT:
                    fetch_w(t + LOOK)
                if t + DQ < NT:
                    dequant(t + DQ)
                if t % XCH == 0 and t // XCH + 2 < NXC:
                    load_xt_chunk(t // XCH + 2)
                if t == 32:
                    # c-term + bias mid-stream (PE warm, off the head and
                    # tail critical paths): out += Xg2[bt,g] @ cT[g,k]
                    for b in range(2):
                        for c in range(2):
                            nc.tensor.matmul(
                                accs[b * 2 + c][:],
                                xg_sb[:, b * 128:(b + 1) * 128],
                                ct_sb[:, c * 512:(c + 1) * 512],
                                start=False, stop=False,
                            )
                ws = ws_ready.pop(t)
                for b in range(2):
                    for c in range(2):
                        nc.tensor.matmul(
                            accs[b * 2 + c][:],
                            xt_sb[:, t, b * 128:(b + 1) * 128],
                            ws[:, c * 512:(c + 1) * 512],
                            start=(t == 0), stop=(t == NT - 1),
                        )

            out_sb = opool.tile([128, 2, KSH], bf16, tag="o")
            out_v = out.ap().rearrange("(b p) k -> p b k", p=128)
            for b in range(2):
                for c in range(2):
                    nc.scalar.copy(out_sb[:, b, c * 512:(c + 1) * 512],
                                   accs[b * 2 + c][:])
                    nc.sync.dma_start(
                        out_v[:, b, c * 512:(c + 1) * 512],
                        out_sb[:, b, c * 512:(c + 1) * 512])

    nc.compile()
    return nc


def _get_nc():
    global _NC_CACHE
    if _NC_CACHE is None:
        _NC_CACHE = _build()
    return _NC_CACHE


def _perm_index():
    # n_of[t, p] = original contraction index held by tile t, partition p
    t = np.arange(NT)[:, None]
    p = np.arange(128)[None, :]
    return (p // 2) * GROUP_SIZE + 2 * t + (p % 2)      # [NT, 128]


def _prep_in_maps(x, W_q, scales, zeros, mu1, mu2, bias):
    import ml_dtypes
    bf16 = ml_dtypes.bfloat16
    x2 = np.asarray(x, dtype=np.float32).reshape(BT, N)
    mu1 = np.asarray(mu1, dtype=np.float32)
    mu2 = np.asarray(mu2, dtype=np.float32)
    bias = np.asarray(bias, dtype=np.float32)
    sc = np.asarray(scales, dtype=np.float32)[:, :, 0]   # [K, NG]
    zr = np.asarray(zeros, dtype=np.float32)[:, :, 0]    # [K, NG]
    W_q = np.asarray(W_q)

    n_of = _perm_index()                                  # [NT, 128]

    xp = x2 * mu1[None, :]                                # x' [BT, N]
    # [NXC=8, 128, XCH=8, BT]: partition-major per chunk
    xt_h = np.ascontiguousarray(
        xp.T[n_of.reshape(-1)].reshape(NT // 8, 8, 128, BT)
        .transpose(0, 2, 1, 3)).astype(bf16)
    Xg = xp.reshape(BT, NG, GROUP_SIZE).sum(axis=2)       # [BT, NG]
    xgt_h = np.concatenate(
        [np.ascontiguousarray(Xg.T), np.ones((1, BT), np.float32)],
        axis=0).astype(bf16)                              # [NG+1, BT]

    a = sc * mu2[:, None]                                 # [K, NG]
    cmat = -zr * a                                        # [K, NG]
    g_of_p = np.arange(128) // 2                          # [128]
    dmask = np.asarray(IS_DIR)

    in_maps = []
    for i in range(NCORES):
        ksl = slice(i * KSH, (i + 1) * KSH)
        wq_core = W_q[ksl, :]                             # [KSH, N] int32
        # [NT, 128, KSH]: tile-major, interleaved rows
        wq_perm = wq_core.T[n_of.reshape(-1)].reshape(NT, 128, KSH)
        srep_h = np.ascontiguousarray(a[ksl, :].T[g_of_p, :]).astype(bf16)
        srep_f = srep_h.astype(np.float32)                # bf16-rounded scales
        wdir_h = np.ascontiguousarray(
            wq_perm[dmask].astype(np.float32) * srep_f[None, :, :]).astype(bf16)
        wq8_h = np.ascontiguousarray(wq_perm[~dmask].astype(np.int8))
        ct_h = np.concatenate(
            [np.ascontiguousarray(cmat[ksl, :].T),
             bias[None, ksl]], axis=0).astype(bf16)       # [NG+1, KSH]
        in_maps.append({"wdir": wdir_h, "wq8": wq8_h, "xt": xt_h,
                        "srep": srep_h, "xgt": xgt_h, "ct": ct_h})
    return in_maps


def _run(inputs, trace=False):
    from concourse import bass_utils
    nc = _get_nc()
    in_maps = _prep_in_maps(**inputs)
    res = bass_utils.run_bass_kernel_spmd(
        nc, in_maps, core_ids=list(range(NCORES)), trace=trace)
    out = np.concatenate([res.results[i]["out"] for i in range(NCORES)],
                         axis=1)                          # [BT, K]
    return out.reshape(B, T, K).astype(np.float32), res


def kernel(**inputs) -> np.ndarray:
    out, _ = _run(inputs, trace=False)
    return out


def kernel_traced(**inputs):
    out, res = _run(inputs, trace=True)
    return out, res
